# revision 50
# baseline (speedup 1.0000x reference)
"""Trainium2 Bass kernel for batch-axis-softmax dot-product attention.

Problem: B=8, S=4096, D=64 fp32.
    scores = einsum('bqd,bkd->bqk', Q, K) / 8
    attn   = softmax(scores, axis=0)          # over the BATCH axis!
    out    = einsum('bqk,bkd->bqd', attn, V)

The batch-axis softmax couples only the 8 batch entries of a fixed (q, k)
position, so sharding over the *query* axis (512 queries per core, K/V
replicated) keeps the softmax fully local to each core.

Per-core pipeline, per k-tile (128 keys x 512 queries, all 8 batches):
  PE : scoresT[k,q] = K_tile @ Q^T   (fp16, fp32 psum; batch pairs packed
       into partition halves -> row-tiled concurrent MMs; each pair's two
       512-wide outputs land in one 2-bank psum tile)
  ACT: E_pair = exp(0.125 * scores_pair)  (one 1024-wide op per pair)
  DVE: Z = sum over the 8 batches (fp16 tree of 5 tensor_adds; GpSimd is
       avoided on purpose — its SBUF-port sharing triples concurrent DVE ops)
  ACT: R = exp(-ln(Z)) = 1/Z, fp16, once per k-tile PAIR at full width
       (ln+exp share the natural_log_exp_and_others table set -> 1 load)
  DVE: W_b = E_b * R  (fp16 2x mode, R pair-broadcast via stride-0 AP)
  PE : outT_b[d,q] += V_tile matmul, accumulated across all 32 k-tiles in
       persistent psum (2 batches per bank via column tiling)
Epilogue: DVE copies psum -> sbuf, DMA to HBM; host reassembles.

Scheduling: a 2-tile-lag software pipeline with AV matmuls drained between
score packs keeps ScalarE (the bottleneck stream, ~97% busy in steady
state) fed. Steady state is ScalarE-bound at ~5.4us per k-tile; HW exec
~196us on 8 cores (vs ~445us for the first working version).
"""

import numpy as np

B = 8
S = 4096
D = 64
NCORES = 8
QBLK = S // NCORES  # 512 queries per core
KT = 128            # keys per k-tile
NKT = S // KT       # 32 k-tiles
NPAIR = B // 2      # batch pairs packed into 128 partitions

# test.py can flip these before calling kernel()
TRACE = False
TRACE_KWARGS = {}
LAST_RESULT = None  # BassKernelResults of the most recent run (for profiling)

_cache = {}


def _build_nc():
    from contextlib import ExitStack

    import concourse.tile as tile
    from concourse import bacc, mybir

    f16 = mybir.dt.float16
    f32 = mybir.dt.float32
    Exp = mybir.ActivationFunctionType.Exp
    Ln = mybir.ActivationFunctionType.Ln

    # Bacc (not raw Bass): its finalize() runs the legalization passes that
    # split multi-wait sync_info into EventSemaphore instructions (TRN2 allows
    # at most one wait per regular instruction).
    #
    # insert_act_table_loads maps each activation func to the first table set
    # containing it, which puts Exp in "exp_and_others" and Ln in
    # "natural_log_exp_and_others" — alternating ACT_TABLE_LOADs every k-tile
    # (~80us of ScalarE). Both funcs live in natural_log_exp_and_others, so
    # restrict Exp/Ln membership to that set: one table load for the whole
    # kernel, hoisted out of the loop.
    class _Bacc(bacc.Bacc):
        def insert_act_table_loads(self):
            from concourse import bass as bass_mod
            from concourse.hw_specs import get_activation_tables

            has_activation = any(
                isinstance(i, mybir.InstActivation)
                for b in self.main_func.blocks
                for i in b.instructions
            )
            if not has_activation:
                return
            combined = "natural_log_exp_and_others"
            tables = []
            for name, fns in get_activation_tables(self.m.arch).items():
                if name != combined:
                    fns = fns - {
                        mybir.ActivationFunctionType.Exp,
                        mybir.ActivationFunctionType.Ln,
                    }
                tables.append((name, fns))
            bass_mod._bass_rust.insert_act_table_loads(self, tables)

    nc = _Bacc()

    # Inputs pre-arranged on host into exact SBUF layouts (fp16):
    #   qt[p, j*512 + q] = Q[2j + p//64, cblk*512 + q, p%64]
    #   kt[p, j*4096 + k] = K[2j + p//64, k, p%64]
    #   vv[p, b*2048 + n*64 + d] = V[b, n*128 + p, d]
    qt_d = nc.dram_tensor("qt", [128, NPAIR * QBLK], f16, kind="ExternalInput")
    kt_d = nc.dram_tensor("kt", [128, NPAIR * S], f16, kind="ExternalInput")
    vv_d = nc.dram_tensor("vv", [128, B * NKT * D], f16, kind="ExternalInput")
    # out[j][(b%2)*64 + d, q] = out_bqd[2j + b%2, q, d]
    out_d = nc.dram_tensor("out", [NPAIR, 128, QBLK], f32, kind="ExternalOutput")

    with tile.TileContext(nc) as tc, ExitStack() as ctx:
        in_p = ctx.enter_context(tc.tile_pool(name="inp", bufs=1))
        e_p = ctx.enter_context(tc.tile_pool(name="e", bufs=20))
        w_p = ctx.enter_context(tc.tile_pool(name="w", bufs=6))
        t_p = ctx.enter_context(tc.tile_pool(name="tree", bufs=3))
        r_p = ctx.enter_context(tc.tile_pool(name="r", bufs=3))
        st_p = ctx.enter_context(tc.tile_pool(name="stage", bufs=1))
        ps_s = ctx.enter_context(tc.tile_pool(name="ps_s", bufs=2, space="PSUM"))
        ps_o = ctx.enter_context(tc.tile_pool(name="ps_o", bufs=1, space="PSUM"))

        # kt/vv are laid out k-tile-major on the host and DMA'd in per-tile
        # chunks interleaved kt/vv, so tile 0's operands land ~7us in and the
        # loop never waits on later chunks.
        qt = in_p.tile([128, NPAIR * QBLK], f16)
        kt = in_p.tile([128, NKT * NPAIR * KT], f16)
        vv = in_p.tile([128, NKT * B * D], f16)
        CH = NPAIR * KT  # 512 columns per k-tile chunk (for both kt and vv)

        def dma_col(dst, src, c0, c1):
            nc.sync.dma_start(out=dst[:, c0:c1], in_=src[:, c0:c1])

        # Issue order: operands of score pack (t=0, j=0) first, then the
        # rest of tile 0, then per-tile chunks so the loop never waits.
        dma_col(qt, qt_d, 0, QBLK)
        dma_col(kt, kt_d, 0, CH)
        for j in range(1, NPAIR):
            dma_col(qt, qt_d, j * QBLK, (j + 1) * QBLK)
        dma_col(vv, vv_d, 0, CH)
        for t in range(1, NKT):
            dma_col(kt, kt_d, t * CH, (t + 1) * CH)
            dma_col(vv, vv_d, t * CH, (t + 1) * CH)

        # Persistent output accumulators: bank j holds batches 2j (parts
        # 0:64) and 2j+1 (parts 64:128), accumulated over all 32 k-tiles.
        oacc = [
            ps_o.tile([128, QBLK], f32, tag=f"oacc{j}", name=f"oacc{j}")
            for j in range(NPAIR)
        ]

        # AV matmuls pending issue; drained between score packs so PE always
        # services the (ACT-feeding) score matmuls promptly instead of
        # running 16-MM AV bursts that starve ScalarE. Interleaving AV MMs
        # of adjacent k-tiles is safe: psum accumulate-adds commute.
        av_pending = []

        def drain_av(n):
            for _ in range(min(n, len(av_pending))):
                av_pending.pop(0)()

        def emit_scores_exp(t):
            # scores + exp, one 2-bank pack per batch pair
            e_packs = []
            for j in range(NPAIR):
                sc = ps_s.tile([128, 2 * QBLK], f32, tag="sc", name=f"sc{t}_{j}")
                for m in range(2):  # m=0 -> b=2j (rows 0:64), m=1 -> b=2j+1
                    rb = m * 64
                    nc.tensor.matmul(
                        out=sc[:, m * QBLK : (m + 1) * QBLK],
                        lhsT=kt[rb : rb + 64, t * CH + j * KT : t * CH + (j + 1) * KT],
                        rhs=qt[rb : rb + 64, j * QBLK : (j + 1) * QBLK],
                        start=True,
                        stop=True,
                        tile_position=(rb, 0),
                    )
                e = e_p.tile([128, 2 * QBLK], f16, tag="e", name=f"e{t}_{j}")
                # E = exp(scores / sqrt(D)); scores*0.125 in [-6, 6] so no
                # max-subtraction is needed and fp16 range is safe.
                nc.scalar.activation(e[:], sc[:], Exp, scale=0.125)
                e_packs.append(e)
                drain_av(2)
            return e_packs

        def emit_tree(t, e_packs):
            # Z = sum_b E_b, all on DVE. GpSimd is deliberately NOT used:
            # any GpSimd tensor op grabs the shared SBUF port and stretches
            # concurrent DVE tensor_tensor ops ~3x (measured 418ns -> 1370ns).
            # Pack-level adds keep the op count at 5 (2 full-width + 3 half).
            t01 = t_p.tile([128, 2 * QBLK], f16, tag="t01", name=f"t01_{t}")
            nc.vector.tensor_add(t01[:], e_packs[0][:], e_packs[1][:])
            t23 = t_p.tile([128, 2 * QBLK], f16, tag="t23", name=f"t23_{t}")
            nc.vector.tensor_add(t23[:], e_packs[2][:], e_packs[3][:])
            u0 = t_p.tile([128, QBLK], f16, tag="u0", name=f"u0_{t}")
            nc.vector.tensor_add(u0[:], t01[:, :QBLK], t01[:, QBLK:])
            u1 = t_p.tile([128, QBLK], f16, tag="u1", name=f"u1_{t}")
            nc.vector.tensor_add(u1[:], t23[:, :QBLK], t23[:, QBLK:])
            # z tiles of consecutive k-tiles share one [128, 1024] pair tile
            # so ln/exp run once per PAIR at full width (less ACT overhead).
            if t % 2 == 0:
                zp = t_p.tile([128, 2 * QBLK], f16, tag="zp", name=f"zp{t}", bufs=3)
                zpairs[t // 2] = zp
            zp = zpairs[t // 2]
            nc.vector.tensor_add(
                zp[:, (t % 2) * QBLK : (t % 2 + 1) * QBLK], u0[:], u1[:]
            )

        def emit_r_pair(tp):
            # R = 1/Z via exp(-ln(Z)) on ScalarE (shared table set), for a
            # pair of k-tiles (2*tp, 2*tp+1) in one full-width op each.
            zp = zpairs.pop(tp)
            lnz = r_p.tile([128, 2 * QBLK], f32, tag="lnz", name=f"lnz{tp}")
            nc.scalar.activation(lnz[:], zp[:], Ln)
            r16 = r_p.tile([128, 2 * QBLK], f16, tag="r16", name=f"r16_{tp}", bufs=3)
            nc.scalar.activation(r16[:], lnz[:], Exp, scale=-1.0)
            return r16

        def emit_mults(t, e_packs, r16, half):
            # W_b = E_b * R, one fp16 2x-mode op per pack with R broadcast
            # over the pair dimension via a stride-0 access pattern.
            r = r16[:, half * QBLK : (half + 1) * QBLK]
            ws = []
            for j in range(NPAIR):
                w = w_p.tile([128, 2 * QBLK], f16, tag="w", name=f"w{t}_{j}")
                nc.vector.tensor_mul(
                    w[:].rearrange("p (a q) -> p a q", a=2),
                    e_packs[j][:].rearrange("p (a q) -> p a q", a=2),
                    r.rearrange("p (a q) -> p a q", a=1).to_broadcast(
                        (128, 2, QBLK)
                    ),
                )
                ws.append(w)
            return ws

        def emit_av(t, ws):
            # outT_b[d,q] += V_b[t]^T-form matmul, queued for interleaved
            # issue (see drain_av). Reverse order so the first-issued AV's
            # wait (on the last mult's DVE tick) covers the others.
            def mk(b):
                j, m = b // 2, b % 2
                rb = m * 64

                def go():
                    nc.tensor.matmul(
                        out=oacc[j][rb : rb + 64, :],
                        lhsT=vv[:, t * CH + b * D : t * CH + (b + 1) * D],
                        rhs=ws[j][:, m * QBLK : (m + 1) * QBLK],
                        start=(t == 0),
                        stop=(t == NKT - 1),
                        tile_position=(0, rb),
                        skip_group_check=True,
                    )

                return go

            for b in reversed(range(B)):
                av_pending.append(mk(b))

        # Software pipeline with a 2-tile lag between the exp/tree front end
        # and the ln/mult/AV back end: every cross-engine input reaching an
        # engine's strict FIFO was produced >= 2 tiles earlier, so waits are
        # pre-satisfied and each engine streams without head-of-line stalls
        # (a 1-tile lag still stalled ACT ~2us per tile on the add-tree).
        # 2-tile-lag software pipeline: the R/mult/AV back end consumes data
        # produced two k-tiles earlier, so its waits are pre-satisfied when
        # they reach each engine's strict FIFO. The back end runs per PAIR of
        # k-tiles (on odd back-tiles) to use full-width ln/exp ops.
        LAG = 2
        zpairs = {}
        state = {}  # t -> e_packs

        def back_end(tp):
            r16 = emit_r_pair(tp)
            for u, half in ((2 * tp, 0), (2 * tp + 1, 1)):
                ws = emit_mults(u, state.pop(u), r16, half)
                emit_av(u, ws)

        for t in range(NKT + LAG):
            tb = t - LAG
            # Back end runs per pair on odd back-tiles; the final pair is
            # pulled one iteration earlier (its z is ready by then). In the
            # last iterations the back end is emitted BEFORE the scores/exps
            # so its R-ops sit ahead of the final exp block in ScalarE's
            # FIFO and the mults/AVs overlap it instead of trailing it.
            do_back = (
                tb >= 0
                and (tb % 2 == 1 or tb == NKT - 2)
                and (tb | 1) // 2 in zpairs
            )
            if t < NKT:
                e_packs = emit_scores_exp(t)
                emit_tree(t, e_packs)
                state[t] = e_packs
            if do_back:
                back_end((tb | 1) // 2)
        drain_av(len(av_pending))

        # One staging tile + one output DMA (4 separate dma_starts each pay
        # ~2us of setup/completion latency in the kernel tail).
        st = st_p.tile([128, NPAIR * QBLK], f32, tag="st")
        for j in range(NPAIR):
            nc.vector.tensor_copy(
                out=st[:, j * QBLK : (j + 1) * QBLK], in_=oacc[j][:]
            )
        nc.sync.dma_start(
            out=out_d[:].rearrange("j p q -> p j q"),
            in_=st[:].rearrange("p (j q) -> p j q", j=NPAIR),
        )

    return nc


def _get_nc():
    if "nc" not in _cache:
        nc = _build_nc()
        if not nc.is_finalized():
            # Runs Bacc.compile() legalization (wait splitting, reg alloc).
            nc.finalize()
        _cache["nc"] = nc
    return _cache["nc"]


def _host_prep(queries, keys, values):
    """Cast to fp16 and pre-arrange into the SBUF layouts (see _build_nc)."""
    k16 = np.asarray(keys, dtype=np.float16)
    v16 = np.asarray(values, dtype=np.float16)
    q16 = np.asarray(queries, dtype=np.float16)

    # kt[(b%2)*64+d, t*512 + (b//2)*128 + kk] = K[b, t*128+kk, d] (k-tile major)
    kt = np.ascontiguousarray(
        k16.reshape(NPAIR, 2, NKT, KT, D)
        .transpose(1, 4, 2, 0, 3)
        .reshape(128, NKT * NPAIR * KT)
    )
    # vv[p, t*512 + b*64 + d] = V[b, t*128+p, d] (k-tile major)
    vv = np.ascontiguousarray(
        v16.reshape(B, NKT, KT, D).transpose(2, 1, 0, 3).reshape(128, NKT * B * D)
    )

    qts = []
    for c in range(NCORES):
        qc = q16[:, c * QBLK : (c + 1) * QBLK, :]  # [8, 512, 64]
        qt = np.ascontiguousarray(
            qc.transpose(0, 2, 1).reshape(NPAIR, 128, QBLK).transpose(1, 0, 2).reshape(128, NPAIR * QBLK)
        )
        qts.append(qt)
    return qts, kt, vv


def kernel(queries, keys, values):
    global LAST_RESULT
    from concourse.bass_utils import run_bass_kernel_spmd

    queries = np.asarray(queries, dtype=np.float32)
    keys = np.asarray(keys, dtype=np.float32)
    values = np.asarray(values, dtype=np.float32)

    nc = _get_nc()
    qts, kt, vv = _host_prep(queries, keys, values)
    in_maps = [{"qt": qts[c], "kt": kt, "vv": vv} for c in range(NCORES)]

    res = run_bass_kernel_spmd(
        nc,
        in_maps,
        list(range(NCORES)),
        trace=TRACE,
        **TRACE_KWARGS,
    )
    LAST_RESULT = res

    out = np.empty((B, S, D), dtype=np.float32)
    for c in range(NCORES):
        o = res.results[c]["out"]  # [4, 128, 512] = [j, (b%2)*64+d, q]
        out[:, c * QBLK : (c + 1) * QBLK, :] = (
            o.reshape(B, D, QBLK).transpose(0, 2, 1)
        )
    return out


# revision 51
# speedup vs baseline: 1.0312x; 1.0312x over previous
"""Trainium2 Bass kernel for batch-axis-softmax dot-product attention.

Problem: B=8, S=4096, D=64 fp32.
    scores = einsum('bqd,bkd->bqk', Q, K) / 8
    attn   = softmax(scores, axis=0)          # over the BATCH axis!
    out    = einsum('bqk,bkd->bqd', attn, V)

The batch-axis softmax couples only the 8 batch entries of a fixed (q, k)
position, so sharding over the *query* axis (512 queries per core, K/V
replicated) keeps the softmax fully local to each core.

Per-core pipeline, per k-tile (128 keys x 512 queries, all 8 batches):
  PE : scoresT[k,q] = K_tile @ Q^T   (fp16, fp32 psum; batch pairs packed
       into partition halves -> row-tiled concurrent MMs; each pair's two
       512-wide outputs land in one 2-bank psum tile)
  ACT: E_pair = exp(0.125 * scores_pair)  (one 1024-wide op per pair)
  DVE: Z = sum over the 8 batches (fp16 tree of 5 tensor_adds; GpSimd is
       avoided on purpose — its SBUF-port sharing triples concurrent DVE ops)
  ACT: R = exp(-ln(Z)) = 1/Z, fp16, once per k-tile PAIR at full width
       (ln+exp share the natural_log_exp_and_others table set -> 1 load)
  DVE: W_b = E_b * R  (fp16 2x mode, R pair-broadcast via stride-0 AP)
  PE : outT_b[d,q] += V_tile matmul, accumulated across all 32 k-tiles in
       persistent psum (2 batches per bank via column tiling)
Epilogue: DVE copies psum -> sbuf, DMA to HBM; host reassembles.

Scheduling: a 2-tile-lag software pipeline with AV matmuls drained between
score packs keeps ScalarE (the bottleneck stream, ~97% busy in steady
state) fed. Steady state is ScalarE-bound at ~5.4us per k-tile; HW exec
~196us on 8 cores (vs ~445us for the first working version).
"""

import numpy as np

B = 8
S = 4096
D = 64
NCORES = 8
QBLK = S // NCORES  # 512 queries per core
KT = 128            # keys per k-tile
NKT = S // KT       # 32 k-tiles
NPAIR = B // 2      # batch pairs packed into 128 partitions

# test.py can flip these before calling kernel()
TRACE = False
TRACE_KWARGS = {}
LAST_RESULT = None  # BassKernelResults of the most recent run (for profiling)

_cache = {}


def _build_nc():
    from contextlib import ExitStack

    import concourse.tile as tile
    from concourse import bacc, mybir

    f16 = mybir.dt.float16
    f32 = mybir.dt.float32
    Exp = mybir.ActivationFunctionType.Exp
    Ln = mybir.ActivationFunctionType.Ln

    # Bacc (not raw Bass): its finalize() runs the legalization passes that
    # split multi-wait sync_info into EventSemaphore instructions (TRN2 allows
    # at most one wait per regular instruction).
    #
    # insert_act_table_loads maps each activation func to the first table set
    # containing it, which puts Exp in "exp_and_others" and Ln in
    # "natural_log_exp_and_others" — alternating ACT_TABLE_LOADs every k-tile
    # (~80us of ScalarE). Both funcs live in natural_log_exp_and_others, so
    # restrict Exp/Ln membership to that set: one table load for the whole
    # kernel, hoisted out of the loop.
    class _Bacc(bacc.Bacc):
        def insert_act_table_loads(self):
            from concourse import bass as bass_mod
            from concourse.hw_specs import get_activation_tables

            has_activation = any(
                isinstance(i, mybir.InstActivation)
                for b in self.main_func.blocks
                for i in b.instructions
            )
            if not has_activation:
                return
            combined = "natural_log_exp_and_others"
            tables = []
            for name, fns in get_activation_tables(self.m.arch).items():
                if name != combined:
                    fns = fns - {
                        mybir.ActivationFunctionType.Exp,
                        mybir.ActivationFunctionType.Ln,
                    }
                tables.append((name, fns))
            bass_mod._bass_rust.insert_act_table_loads(self, tables)

    nc = _Bacc()

    # Inputs pre-arranged on host into exact SBUF layouts (fp16):
    #   qt[p, j*512 + q] = Q[2j + p//64, cblk*512 + q, p%64]
    #   kt[p, j*4096 + k] = K[2j + p//64, k, p%64]
    #   vv[p, b*2048 + n*64 + d] = V[b, n*128 + p, d]
    qt_d = nc.dram_tensor("qt", [128, NPAIR * QBLK], f16, kind="ExternalInput")
    kt_d = nc.dram_tensor("kt", [128, NPAIR * S], f16, kind="ExternalInput")
    vv_d = nc.dram_tensor("vv", [128, B * NKT * D], f16, kind="ExternalInput")
    # out[j][(b%2)*64 + d, q] = out_bqd[2j + b%2, q, d]
    out_d = nc.dram_tensor("out", [NPAIR, 128, QBLK], f32, kind="ExternalOutput")

    with tile.TileContext(nc) as tc, ExitStack() as ctx:
        in_p = ctx.enter_context(tc.tile_pool(name="inp", bufs=1))
        e_p = ctx.enter_context(tc.tile_pool(name="e", bufs=17))
        w_p = ctx.enter_context(tc.tile_pool(name="w", bufs=6))
        t_p = ctx.enter_context(tc.tile_pool(name="tree", bufs=3))
        r_p = ctx.enter_context(tc.tile_pool(name="r", bufs=2))
        st_p = ctx.enter_context(tc.tile_pool(name="stage", bufs=1))
        ps_s = ctx.enter_context(tc.tile_pool(name="ps_s", bufs=2, space="PSUM"))
        ps_o = ctx.enter_context(tc.tile_pool(name="ps_o", bufs=1, space="PSUM"))

        # kt/vv are laid out k-tile-major on the host and DMA'd in per-tile
        # chunks interleaved kt/vv, so tile 0's operands land ~7us in and the
        # loop never waits on later chunks.
        qt = in_p.tile([128, NPAIR * QBLK], f16)
        kt = in_p.tile([128, NKT * NPAIR * KT], f16)
        vv = in_p.tile([128, NKT * B * D], f16)
        CH = NPAIR * KT  # 512 columns per k-tile chunk (for both kt and vv)

        def dma_col(dst, src, c0, c1):
            nc.sync.dma_start(out=dst[:, c0:c1], in_=src[:, c0:c1])

        # Issue order: operands of score pack (t=0, j=0) first, then the
        # rest of tile 0, then per-tile chunks so the loop never waits.
        dma_col(qt, qt_d, 0, QBLK)
        dma_col(kt, kt_d, 0, CH)
        for j in range(1, NPAIR):
            dma_col(qt, qt_d, j * QBLK, (j + 1) * QBLK)
        dma_col(vv, vv_d, 0, CH)
        for t in range(1, NKT):
            dma_col(kt, kt_d, t * CH, (t + 1) * CH)
            dma_col(vv, vv_d, t * CH, (t + 1) * CH)

        # Persistent output accumulators: bank j holds batches 2j (parts
        # 0:64) and 2j+1 (parts 64:128), accumulated over all 32 k-tiles.
        oacc = [
            ps_o.tile([128, QBLK], f32, tag=f"oacc{j}", name=f"oacc{j}")
            for j in range(NPAIR)
        ]

        # AV matmuls pending issue; drained between score packs so PE always
        # services the (ACT-feeding) score matmuls promptly instead of
        # running 16-MM AV bursts that starve ScalarE. Interleaving AV MMs
        # of adjacent k-tiles is safe: psum accumulate-adds commute.
        av_pending = []

        def drain_av(n):
            for _ in range(min(n, len(av_pending))):
                av_pending.pop(0)()

        def emit_scores_exp(t):
            # scores + exp, one 2-bank pack per batch pair
            e_packs = []
            for j in range(NPAIR):
                sc = ps_s.tile([128, 2 * QBLK], f32, tag="sc", name=f"sc{t}_{j}")
                for m in range(2):  # m=0 -> b=2j (rows 0:64), m=1 -> b=2j+1
                    rb = m * 64
                    nc.tensor.matmul(
                        out=sc[:, m * QBLK : (m + 1) * QBLK],
                        lhsT=kt[rb : rb + 64, t * CH + j * KT : t * CH + (j + 1) * KT],
                        rhs=qt[rb : rb + 64, j * QBLK : (j + 1) * QBLK],
                        start=True,
                        stop=True,
                        tile_position=(rb, 0),
                    )
                e = e_p.tile([128, 2 * QBLK], f16, tag="e", name=f"e{t}_{j}")
                # E = exp(scores / sqrt(D)); scores*0.125 in [-6, 6] so no
                # max-subtraction is needed and fp16 range is safe.
                nc.scalar.activation(e[:], sc[:], Exp, scale=0.125)
                e_packs.append(e)
                drain_av(2)
            return e_packs

        def emit_tree(t, e_packs):
            # Z = sum_b E_b, all on DVE. GpSimd is deliberately NOT used:
            # any GpSimd tensor op grabs the shared SBUF port and stretches
            # concurrent DVE tensor_tensor ops ~3x (measured 418ns -> 1370ns).
            # Pack-level adds keep the op count at 5 (2 full-width + 3 half).
            t01 = t_p.tile([128, 2 * QBLK], f16, tag="t01", name=f"t01_{t}")
            nc.vector.tensor_add(t01[:], e_packs[0][:], e_packs[1][:])
            t23 = t_p.tile([128, 2 * QBLK], f16, tag="t23", name=f"t23_{t}")
            nc.vector.tensor_add(t23[:], e_packs[2][:], e_packs[3][:])
            u0 = t_p.tile([128, QBLK], f16, tag="u0", name=f"u0_{t}")
            nc.vector.tensor_add(u0[:], t01[:, :QBLK], t01[:, QBLK:])
            u1 = t_p.tile([128, QBLK], f16, tag="u1", name=f"u1_{t}")
            nc.vector.tensor_add(u1[:], t23[:, :QBLK], t23[:, QBLK:])
            # z tiles of consecutive k-tiles share one [128, 1024] pair tile
            # so ln/exp run once per PAIR at full width (less ACT overhead).
            if t % 2 == 0:
                zp = t_p.tile([128, 2 * QBLK], f16, tag="zp", name=f"zp{t}", bufs=3)
                zpairs[t // 2] = zp
            zp = zpairs[t // 2]
            nc.vector.tensor_add(
                zp[:, (t % 2) * QBLK : (t % 2 + 1) * QBLK], u0[:], u1[:]
            )

        def emit_r_pair(tp):
            # R = 1/Z via exp(-ln(Z)) on ScalarE (shared table set), for a
            # pair of k-tiles (2*tp, 2*tp+1) in one full-width op each.
            zp = zpairs.pop(tp)
            lnz = r_p.tile([128, 2 * QBLK], f32, tag="lnz", name=f"lnz{tp}")
            nc.scalar.activation(lnz[:], zp[:], Ln)
            r16 = r_p.tile([128, 2 * QBLK], f16, tag="r16", name=f"r16_{tp}", bufs=3)
            nc.scalar.activation(r16[:], lnz[:], Exp, scale=-1.0)
            return r16

        def emit_mults(t, e_packs, r16, half):
            # W_b = E_b * R, one fp16 2x-mode op per pack with R broadcast
            # over the pair dimension via a stride-0 access pattern.
            r = r16[:, half * QBLK : (half + 1) * QBLK]
            ws = []
            for j in range(NPAIR):
                w = w_p.tile([128, 2 * QBLK], f16, tag="w", name=f"w{t}_{j}")
                nc.vector.tensor_mul(
                    w[:].rearrange("p (a q) -> p a q", a=2),
                    e_packs[j][:].rearrange("p (a q) -> p a q", a=2),
                    r.rearrange("p (a q) -> p a q", a=1).to_broadcast(
                        (128, 2, QBLK)
                    ),
                )
                ws.append(w)
            return ws

        def emit_av(t, ws):
            # outT_b[d,q] += V_b[t]^T-form matmul, queued for interleaved
            # issue (see drain_av). Reverse order so the first-issued AV's
            # wait (on the last mult's DVE tick) covers the others.
            def mk(b):
                j, m = b // 2, b % 2
                rb = m * 64

                def go():
                    nc.tensor.matmul(
                        out=oacc[j][rb : rb + 64, :],
                        lhsT=vv[:, t * CH + b * D : t * CH + (b + 1) * D],
                        rhs=ws[j][:, m * QBLK : (m + 1) * QBLK],
                        start=(t == 0),
                        stop=(t == NKT - 1),
                        tile_position=(0, rb),
                        skip_group_check=True,
                    )

                return go

            for b in reversed(range(B)):
                av_pending.append(mk(b))

        # Software pipeline with a 2-tile lag between the exp/tree front end
        # and the ln/mult/AV back end: every cross-engine input reaching an
        # engine's strict FIFO was produced >= 2 tiles earlier, so waits are
        # pre-satisfied and each engine streams without head-of-line stalls
        # (a 1-tile lag still stalled ACT ~2us per tile on the add-tree).
        # 2-tile-lag software pipeline: the R/mult/AV back end consumes data
        # produced two k-tiles earlier, so its waits are pre-satisfied when
        # they reach each engine's strict FIFO. The back end runs per PAIR of
        # k-tiles (on odd back-tiles) to use full-width ln/exp ops.
        LAG = 2
        zpairs = {}
        state = {}  # t -> e_packs

        def back_end(tp):
            r16 = emit_r_pair(tp)
            for u, half in ((2 * tp, 0), (2 * tp + 1, 1)):
                ws = emit_mults(u, state.pop(u), r16, half)
                emit_av(u, ws)

        for t in range(NKT + LAG):
            tb = t - LAG
            # Back end runs per pair on odd back-tiles; the final pair is
            # pulled one iteration earlier (its z is ready by then). In the
            # last iterations the back end is emitted BEFORE the scores/exps
            # so its R-ops sit ahead of the final exp block in ScalarE's
            # FIFO and the mults/AVs overlap it instead of trailing it.
            do_back = (
                tb >= 0
                and (tb % 2 == 1 or tb == NKT - 2)
                and (tb | 1) // 2 in zpairs
            )
            if t < NKT:
                e_packs = emit_scores_exp(t)
                emit_tree(t, e_packs)
                state[t] = e_packs
            if do_back:
                back_end((tb | 1) // 2)
        drain_av(len(av_pending))

        # One staging tile + one output DMA (4 separate dma_starts each pay
        # ~2us of setup/completion latency in the kernel tail).
        st = st_p.tile([128, NPAIR * QBLK], f32, tag="st")
        for j in range(NPAIR):
            nc.vector.tensor_copy(
                out=st[:, j * QBLK : (j + 1) * QBLK], in_=oacc[j][:]
            )
        nc.sync.dma_start(
            out=out_d[:].rearrange("j p q -> p j q"),
            in_=st[:].rearrange("p (j q) -> p j q", j=NPAIR),
        )

    return nc


def _get_nc():
    if "nc" not in _cache:
        nc = _build_nc()
        if not nc.is_finalized():
            # Runs Bacc.compile() legalization (wait splitting, reg alloc).
            nc.finalize()
        _cache["nc"] = nc
    return _cache["nc"]


def _host_prep(queries, keys, values):
    """Cast to fp16 and pre-arrange into the SBUF layouts (see _build_nc)."""
    k16 = np.asarray(keys, dtype=np.float16)
    v16 = np.asarray(values, dtype=np.float16)
    q16 = np.asarray(queries, dtype=np.float16)

    # kt[(b%2)*64+d, t*512 + (b//2)*128 + kk] = K[b, t*128+kk, d] (k-tile major)
    kt = np.ascontiguousarray(
        k16.reshape(NPAIR, 2, NKT, KT, D)
        .transpose(1, 4, 2, 0, 3)
        .reshape(128, NKT * NPAIR * KT)
    )
    # vv[p, t*512 + b*64 + d] = V[b, t*128+p, d] (k-tile major)
    vv = np.ascontiguousarray(
        v16.reshape(B, NKT, KT, D).transpose(2, 1, 0, 3).reshape(128, NKT * B * D)
    )

    qts = []
    for c in range(NCORES):
        qc = q16[:, c * QBLK : (c + 1) * QBLK, :]  # [8, 512, 64]
        qt = np.ascontiguousarray(
            qc.transpose(0, 2, 1).reshape(NPAIR, 128, QBLK).transpose(1, 0, 2).reshape(128, NPAIR * QBLK)
        )
        qts.append(qt)
    return qts, kt, vv


def kernel(queries, keys, values):
    global LAST_RESULT
    from concourse.bass_utils import run_bass_kernel_spmd

    queries = np.asarray(queries, dtype=np.float32)
    keys = np.asarray(keys, dtype=np.float32)
    values = np.asarray(values, dtype=np.float32)

    nc = _get_nc()
    qts, kt, vv = _host_prep(queries, keys, values)
    in_maps = [{"qt": qts[c], "kt": kt, "vv": vv} for c in range(NCORES)]

    res = run_bass_kernel_spmd(
        nc,
        in_maps,
        list(range(NCORES)),
        trace=TRACE,
        **TRACE_KWARGS,
    )
    LAST_RESULT = res

    out = np.empty((B, S, D), dtype=np.float32)
    for c in range(NCORES):
        o = res.results[c]["out"]  # [4, 128, 512] = [j, (b%2)*64+d, q]
        out[:, c * QBLK : (c + 1) * QBLK, :] = (
            o.reshape(B, D, QBLK).transpose(0, 2, 1)
        )
    return out


# revision 52
# speedup vs baseline: 1.0322x; 1.0010x over previous
"""Trainium2 Bass kernel for batch-axis-softmax dot-product attention.

Problem: B=8, S=4096, D=64 fp32.
    scores = einsum('bqd,bkd->bqk', Q, K) / 8
    attn   = softmax(scores, axis=0)          # over the BATCH axis!
    out    = einsum('bqk,bkd->bqd', attn, V)

The batch-axis softmax couples only the 8 batch entries of a fixed (q, k)
position, so sharding over the *query* axis (512 queries per core, K/V
replicated) keeps the softmax fully local to each core.

Per-core pipeline, per k-tile (128 keys x 512 queries, all 8 batches):
  PE : scoresT[k,q] = K_tile @ Q^T   (fp16, fp32 psum; batch pairs packed
       into partition halves -> row-tiled concurrent MMs; each pair's two
       512-wide outputs land in one 2-bank psum tile)
  ACT: E_pair = exp(0.125 * scores_pair)  (one 1024-wide op per pair)
  DVE: Z = sum over the 8 batches (fp16 tree of 5 tensor_adds; GpSimd is
       avoided on purpose — its SBUF-port sharing triples concurrent DVE ops)
  ACT: R = exp(-ln(Z)) = 1/Z, fp16, once per k-tile PAIR at full width
       (ln+exp share the natural_log_exp_and_others table set -> 1 load)
  DVE: W_b = E_b * R  (fp16 2x mode, R pair-broadcast via stride-0 AP)
  PE : outT_b[d,q] += V_tile matmul, accumulated across all 32 k-tiles in
       persistent psum (2 batches per bank via column tiling)
Epilogue: DVE copies psum -> sbuf, DMA to HBM; host reassembles.

Scheduling: a 2-tile-lag software pipeline with AV matmuls drained between
score packs keeps ScalarE (the bottleneck stream, ~97% busy in steady
state) fed. Steady state is ScalarE-bound at ~5.4us per k-tile; HW exec
~196us on 8 cores (vs ~445us for the first working version).
"""

import numpy as np

B = 8
S = 4096
D = 64
NCORES = 8
QBLK = S // NCORES  # 512 queries per core
KT = 128            # keys per k-tile
NKT = S // KT       # 32 k-tiles
NPAIR = B // 2      # batch pairs packed into 128 partitions

# test.py can flip these before calling kernel()
TRACE = False
TRACE_KWARGS = {}
LAST_RESULT = None  # BassKernelResults of the most recent run (for profiling)

_cache = {}


def _build_nc():
    from contextlib import ExitStack

    import concourse.tile as tile
    from concourse import bacc, mybir

    f16 = mybir.dt.float16
    f32 = mybir.dt.float32
    Exp = mybir.ActivationFunctionType.Exp
    Ln = mybir.ActivationFunctionType.Ln

    # Bacc (not raw Bass): its finalize() runs the legalization passes that
    # split multi-wait sync_info into EventSemaphore instructions (TRN2 allows
    # at most one wait per regular instruction).
    #
    # insert_act_table_loads maps each activation func to the first table set
    # containing it, which puts Exp in "exp_and_others" and Ln in
    # "natural_log_exp_and_others" — alternating ACT_TABLE_LOADs every k-tile
    # (~80us of ScalarE). Both funcs live in natural_log_exp_and_others, so
    # restrict Exp/Ln membership to that set: one table load for the whole
    # kernel, hoisted out of the loop.
    class _Bacc(bacc.Bacc):
        def insert_act_table_loads(self):
            from concourse import bass as bass_mod
            from concourse.hw_specs import get_activation_tables

            has_activation = any(
                isinstance(i, mybir.InstActivation)
                for b in self.main_func.blocks
                for i in b.instructions
            )
            if not has_activation:
                return
            combined = "natural_log_exp_and_others"
            tables = []
            for name, fns in get_activation_tables(self.m.arch).items():
                if name != combined:
                    fns = fns - {
                        mybir.ActivationFunctionType.Exp,
                        mybir.ActivationFunctionType.Ln,
                    }
                tables.append((name, fns))
            bass_mod._bass_rust.insert_act_table_loads(self, tables)

    nc = _Bacc()

    # Inputs pre-arranged on host into exact SBUF layouts (fp16):
    #   qt[p, j*512 + q] = Q[2j + p//64, cblk*512 + q, p%64]
    #   kt[p, j*4096 + k] = K[2j + p//64, k, p%64]
    #   vv[p, b*2048 + n*64 + d] = V[b, n*128 + p, d]
    qt_d = nc.dram_tensor("qt", [128, NPAIR * QBLK], f16, kind="ExternalInput")
    kt_d = nc.dram_tensor("kt", [128, NPAIR * S], f16, kind="ExternalInput")
    vv_d = nc.dram_tensor("vv", [128, B * NKT * D], f16, kind="ExternalInput")
    # out[j][(b%2)*64 + d, q] = out_bqd[2j + b%2, q, d]
    out_d = nc.dram_tensor("out", [NPAIR, 128, QBLK], f32, kind="ExternalOutput")

    with tile.TileContext(nc) as tc, ExitStack() as ctx:
        in_p = ctx.enter_context(tc.tile_pool(name="inp", bufs=1))
        e_p = ctx.enter_context(tc.tile_pool(name="e", bufs=16))
        w_p = ctx.enter_context(tc.tile_pool(name="w", bufs=6))
        t_p = ctx.enter_context(tc.tile_pool(name="tree", bufs=3))
        r_p = ctx.enter_context(tc.tile_pool(name="r", bufs=2))
        st_p = ctx.enter_context(tc.tile_pool(name="stage", bufs=1))
        ps_s = ctx.enter_context(tc.tile_pool(name="ps_s", bufs=2, space="PSUM"))
        ps_o = ctx.enter_context(tc.tile_pool(name="ps_o", bufs=1, space="PSUM"))

        # kt/vv are laid out k-tile-major on the host and DMA'd in per-tile
        # chunks interleaved kt/vv, so tile 0's operands land ~7us in and the
        # loop never waits on later chunks.
        qt = in_p.tile([128, NPAIR * QBLK], f16)
        kt = in_p.tile([128, NKT * NPAIR * KT], f16)
        vv = in_p.tile([128, NKT * B * D], f16)
        CH = NPAIR * KT  # 512 columns per k-tile chunk (for both kt and vv)

        def dma_col(dst, src, c0, c1):
            nc.sync.dma_start(out=dst[:, c0:c1], in_=src[:, c0:c1])

        # Issue order: operands of score pack (t=0, j=0) first, then the
        # rest of tile 0, then per-tile chunks so the loop never waits.
        dma_col(qt, qt_d, 0, QBLK)
        dma_col(kt, kt_d, 0, CH)
        for j in range(1, NPAIR):
            dma_col(qt, qt_d, j * QBLK, (j + 1) * QBLK)
        dma_col(vv, vv_d, 0, CH)
        for t in range(1, NKT):
            dma_col(kt, kt_d, t * CH, (t + 1) * CH)
            dma_col(vv, vv_d, t * CH, (t + 1) * CH)

        # Persistent output accumulators: bank j holds batches 2j (parts
        # 0:64) and 2j+1 (parts 64:128), accumulated over all 32 k-tiles.
        oacc = [
            ps_o.tile([128, QBLK], f32, tag=f"oacc{j}", name=f"oacc{j}")
            for j in range(NPAIR)
        ]

        # AV matmuls pending issue; drained between score packs so PE always
        # services the (ACT-feeding) score matmuls promptly instead of
        # running 16-MM AV bursts that starve ScalarE. Interleaving AV MMs
        # of adjacent k-tiles is safe: psum accumulate-adds commute.
        av_pending = []

        def drain_av(n):
            for _ in range(min(n, len(av_pending))):
                av_pending.pop(0)()

        def emit_scores_exp(t):
            # scores + exp, one 2-bank pack per batch pair
            e_packs = []
            for j in range(NPAIR):
                sc = ps_s.tile([128, 2 * QBLK], f32, tag="sc", name=f"sc{t}_{j}")
                for m in range(2):  # m=0 -> b=2j (rows 0:64), m=1 -> b=2j+1
                    rb = m * 64
                    nc.tensor.matmul(
                        out=sc[:, m * QBLK : (m + 1) * QBLK],
                        lhsT=kt[rb : rb + 64, t * CH + j * KT : t * CH + (j + 1) * KT],
                        rhs=qt[rb : rb + 64, j * QBLK : (j + 1) * QBLK],
                        start=True,
                        stop=True,
                        tile_position=(rb, 0),
                    )
                e = e_p.tile([128, 2 * QBLK], f16, tag="e", name=f"e{t}_{j}")
                # E = exp(scores / sqrt(D)); scores*0.125 in [-6, 6] so no
                # max-subtraction is needed and fp16 range is safe.
                nc.scalar.activation(e[:], sc[:], Exp, scale=0.125)
                e_packs.append(e)
                drain_av(2)
            return e_packs

        def emit_tree(t, e_packs):
            # Z = sum_b E_b, all on DVE. GpSimd is deliberately NOT used:
            # any GpSimd tensor op grabs the shared SBUF port and stretches
            # concurrent DVE tensor_tensor ops ~3x (measured 418ns -> 1370ns).
            # Pack-level adds keep the op count at 5 (2 full-width + 3 half).
            t01 = t_p.tile([128, 2 * QBLK], f16, tag="t01", name=f"t01_{t}")
            nc.vector.tensor_add(t01[:], e_packs[0][:], e_packs[1][:])
            t23 = t_p.tile([128, 2 * QBLK], f16, tag="t23", name=f"t23_{t}")
            nc.vector.tensor_add(t23[:], e_packs[2][:], e_packs[3][:])
            u0 = t_p.tile([128, QBLK], f16, tag="u0", name=f"u0_{t}")
            nc.vector.tensor_add(u0[:], t01[:, :QBLK], t01[:, QBLK:])
            u1 = t_p.tile([128, QBLK], f16, tag="u1", name=f"u1_{t}")
            nc.vector.tensor_add(u1[:], t23[:, :QBLK], t23[:, QBLK:])
            # z tiles of consecutive k-tiles share one [128, 1024] pair tile
            # so ln/exp run once per PAIR at full width (less ACT overhead).
            if t % 2 == 0:
                zp = t_p.tile([128, 2 * QBLK], f16, tag="zp", name=f"zp{t}", bufs=3)
                zpairs[t // 2] = zp
            zp = zpairs[t // 2]
            nc.vector.tensor_add(
                zp[:, (t % 2) * QBLK : (t % 2 + 1) * QBLK], u0[:], u1[:]
            )

        def emit_r_pair(tp):
            # R = 1/Z via exp(-ln(Z)) on ScalarE (shared table set), for a
            # pair of k-tiles (2*tp, 2*tp+1) in one full-width op each.
            zp = zpairs.pop(tp)
            lnz = r_p.tile([128, 2 * QBLK], f32, tag="lnz", name=f"lnz{tp}")
            nc.scalar.activation(lnz[:], zp[:], Ln)
            r16 = r_p.tile([128, 2 * QBLK], f16, tag="r16", name=f"r16_{tp}", bufs=3)
            nc.scalar.activation(r16[:], lnz[:], Exp, scale=-1.0)
            return r16

        def emit_mults(t, e_packs, r16, half):
            # W_b = E_b * R, one fp16 2x-mode op per pack with R broadcast
            # over the pair dimension via a stride-0 access pattern.
            r = r16[:, half * QBLK : (half + 1) * QBLK]
            ws = []
            for j in range(NPAIR):
                w = w_p.tile([128, 2 * QBLK], f16, tag="w", name=f"w{t}_{j}")
                nc.vector.tensor_mul(
                    w[:].rearrange("p (a q) -> p a q", a=2),
                    e_packs[j][:].rearrange("p (a q) -> p a q", a=2),
                    r.rearrange("p (a q) -> p a q", a=1).to_broadcast(
                        (128, 2, QBLK)
                    ),
                )
                ws.append(w)
            return ws

        def emit_av(t, ws):
            # outT_b[d,q] += V_b[t]^T-form matmul, queued for interleaved
            # issue (see drain_av). Reverse order so the first-issued AV's
            # wait (on the last mult's DVE tick) covers the others.
            def mk(b):
                j, m = b // 2, b % 2
                rb = m * 64

                def go():
                    nc.tensor.matmul(
                        out=oacc[j][rb : rb + 64, :],
                        lhsT=vv[:, t * CH + b * D : t * CH + (b + 1) * D],
                        rhs=ws[j][:, m * QBLK : (m + 1) * QBLK],
                        start=(t == 0),
                        stop=(t == NKT - 1),
                        tile_position=(0, rb),
                        skip_group_check=True,
                    )

                return go

            for b in reversed(range(B)):
                av_pending.append(mk(b))

        # Software pipeline with a 2-tile lag between the exp/tree front end
        # and the ln/mult/AV back end: every cross-engine input reaching an
        # engine's strict FIFO was produced >= 2 tiles earlier, so waits are
        # pre-satisfied and each engine streams without head-of-line stalls
        # (a 1-tile lag still stalled ACT ~2us per tile on the add-tree).
        # 2-tile-lag software pipeline: the R/mult/AV back end consumes data
        # produced two k-tiles earlier, so its waits are pre-satisfied when
        # they reach each engine's strict FIFO. The back end runs per PAIR of
        # k-tiles (on odd back-tiles) to use full-width ln/exp ops.
        LAG = 2
        zpairs = {}
        state = {}  # t -> e_packs

        def back_end(tp):
            r16 = emit_r_pair(tp)
            for u, half in ((2 * tp, 0), (2 * tp + 1, 1)):
                ws = emit_mults(u, state.pop(u), r16, half)
                emit_av(u, ws)

        for t in range(NKT + LAG):
            tb = t - LAG
            # Back end runs per pair on odd back-tiles; the final pair is
            # pulled one iteration earlier (its z is ready by then). In the
            # last iterations the back end is emitted BEFORE the scores/exps
            # so its R-ops sit ahead of the final exp block in ScalarE's
            # FIFO and the mults/AVs overlap it instead of trailing it.
            do_back = (
                tb >= 0
                and (tb % 2 == 1 or tb == NKT - 2)
                and (tb | 1) // 2 in zpairs
            )
            if t < NKT:
                e_packs = emit_scores_exp(t)
                emit_tree(t, e_packs)
                state[t] = e_packs
            if do_back:
                back_end((tb | 1) // 2)
        drain_av(len(av_pending))

        # One staging tile + one output DMA (4 separate dma_starts each pay
        # ~2us of setup/completion latency in the kernel tail).
        st = st_p.tile([128, NPAIR * QBLK], f32, tag="st")
        for j in range(NPAIR):
            nc.vector.tensor_copy(
                out=st[:, j * QBLK : (j + 1) * QBLK], in_=oacc[j][:]
            )
        nc.sync.dma_start(
            out=out_d[:].rearrange("j p q -> p j q"),
            in_=st[:].rearrange("p (j q) -> p j q", j=NPAIR),
        )

    return nc


def _get_nc():
    if "nc" not in _cache:
        nc = _build_nc()
        if not nc.is_finalized():
            # Runs Bacc.compile() legalization (wait splitting, reg alloc).
            nc.finalize()
        _cache["nc"] = nc
    return _cache["nc"]


def _host_prep(queries, keys, values):
    """Cast to fp16 and pre-arrange into the SBUF layouts (see _build_nc)."""
    k16 = np.asarray(keys, dtype=np.float16)
    v16 = np.asarray(values, dtype=np.float16)
    q16 = np.asarray(queries, dtype=np.float16)

    # kt[(b%2)*64+d, t*512 + (b//2)*128 + kk] = K[b, t*128+kk, d] (k-tile major)
    kt = np.ascontiguousarray(
        k16.reshape(NPAIR, 2, NKT, KT, D)
        .transpose(1, 4, 2, 0, 3)
        .reshape(128, NKT * NPAIR * KT)
    )
    # vv[p, t*512 + b*64 + d] = V[b, t*128+p, d] (k-tile major)
    vv = np.ascontiguousarray(
        v16.reshape(B, NKT, KT, D).transpose(2, 1, 0, 3).reshape(128, NKT * B * D)
    )

    qts = []
    for c in range(NCORES):
        qc = q16[:, c * QBLK : (c + 1) * QBLK, :]  # [8, 512, 64]
        qt = np.ascontiguousarray(
            qc.transpose(0, 2, 1).reshape(NPAIR, 128, QBLK).transpose(1, 0, 2).reshape(128, NPAIR * QBLK)
        )
        qts.append(qt)
    return qts, kt, vv


def kernel(queries, keys, values):
    global LAST_RESULT
    from concourse.bass_utils import run_bass_kernel_spmd

    queries = np.asarray(queries, dtype=np.float32)
    keys = np.asarray(keys, dtype=np.float32)
    values = np.asarray(values, dtype=np.float32)

    nc = _get_nc()
    qts, kt, vv = _host_prep(queries, keys, values)
    in_maps = [{"qt": qts[c], "kt": kt, "vv": vv} for c in range(NCORES)]

    res = run_bass_kernel_spmd(
        nc,
        in_maps,
        list(range(NCORES)),
        trace=TRACE,
        **TRACE_KWARGS,
    )
    LAST_RESULT = res

    out = np.empty((B, S, D), dtype=np.float32)
    for c in range(NCORES):
        o = res.results[c]["out"]  # [4, 128, 512] = [j, (b%2)*64+d, q]
        out[:, c * QBLK : (c + 1) * QBLK, :] = (
            o.reshape(B, D, QBLK).transpose(0, 2, 1)
        )
    return out


# revision 53
# speedup vs baseline: 1.0432x; 1.0106x over previous
"""Trainium2 Bass kernel for batch-axis-softmax dot-product attention.

Problem: B=8, S=4096, D=64 fp32.
    scores = einsum('bqd,bkd->bqk', Q, K) / 8
    attn   = softmax(scores, axis=0)          # over the BATCH axis!
    out    = einsum('bqk,bkd->bqd', attn, V)

The batch-axis softmax couples only the 8 batch entries of a fixed (q, k)
position, so sharding over the *query* axis (512 queries per core, K/V
replicated) keeps the softmax fully local to each core.

Per-core pipeline, per k-tile (128 keys x 512 queries, all 8 batches):
  PE : scoresT[k,q] = K_tile @ Q^T   (fp16, fp32 psum; batch pairs packed
       into partition halves -> row-tiled concurrent MMs; each pair's two
       512-wide outputs land in one 2-bank psum tile)
  ACT: E_pair = exp(0.125 * scores_pair)  (one 1024-wide op per pair)
  DVE: Z = sum over the 8 batches (fp16 tree of 5 tensor_adds; GpSimd is
       avoided on purpose — its SBUF-port sharing triples concurrent DVE ops)
  ACT: R = exp(-ln(Z)) = 1/Z, fp16, once per k-tile PAIR at full width
       (ln+exp share the natural_log_exp_and_others table set -> 1 load)
  DVE: W_b = E_b * R  (fp16 2x mode, R pair-broadcast via stride-0 AP)
  PE : outT_b[d,q] += V_tile matmul, accumulated across all 32 k-tiles in
       persistent psum (2 batches per bank via column tiling)
Epilogue: DVE copies psum -> sbuf, DMA to HBM; host reassembles.

Scheduling: a 2-tile-lag software pipeline with AV matmuls drained between
score packs keeps ScalarE (the bottleneck stream, ~97% busy in steady
state) fed. Steady state is ScalarE-bound at ~5.4us per k-tile; HW exec
~196us on 8 cores (vs ~445us for the first working version).
"""

import numpy as np

B = 8
S = 4096
D = 64
NCORES = 8
QBLK = S // NCORES  # 512 queries per core
KT = 128            # keys per k-tile
NKT = S // KT       # 32 k-tiles
NPAIR = B // 2      # batch pairs packed into 128 partitions

# test.py can flip these before calling kernel()
TRACE = False
TRACE_KWARGS = {}
LAST_RESULT = None  # BassKernelResults of the most recent run (for profiling)

_cache = {}


def _build_nc():
    from contextlib import ExitStack

    import concourse.tile as tile
    from concourse import bacc, mybir

    f16 = mybir.dt.float16
    f32 = mybir.dt.float32
    Exp = mybir.ActivationFunctionType.Exp
    Ln = mybir.ActivationFunctionType.Ln

    # Bacc (not raw Bass): its finalize() runs the legalization passes that
    # split multi-wait sync_info into EventSemaphore instructions (TRN2 allows
    # at most one wait per regular instruction).
    #
    # insert_act_table_loads maps each activation func to the first table set
    # containing it, which puts Exp in "exp_and_others" and Ln in
    # "natural_log_exp_and_others" — alternating ACT_TABLE_LOADs every k-tile
    # (~80us of ScalarE). Both funcs live in natural_log_exp_and_others, so
    # restrict Exp/Ln membership to that set: one table load for the whole
    # kernel, hoisted out of the loop.
    class _Bacc(bacc.Bacc):
        def insert_act_table_loads(self):
            from concourse import bass as bass_mod
            from concourse.hw_specs import get_activation_tables

            has_activation = any(
                isinstance(i, mybir.InstActivation)
                for b in self.main_func.blocks
                for i in b.instructions
            )
            if not has_activation:
                return
            combined = "natural_log_exp_and_others"
            tables = []
            for name, fns in get_activation_tables(self.m.arch).items():
                if name != combined:
                    fns = fns - {
                        mybir.ActivationFunctionType.Exp,
                        mybir.ActivationFunctionType.Ln,
                    }
                tables.append((name, fns))
            bass_mod._bass_rust.insert_act_table_loads(self, tables)

    nc = _Bacc()

    # Inputs pre-arranged on host into exact SBUF layouts (fp16):
    #   qt[p, j*512 + q] = Q[2j + p//64, cblk*512 + q, p%64]
    #   kt[p, j*4096 + k] = K[2j + p//64, k, p%64]
    #   vv[p, b*2048 + n*64 + d] = V[b, n*128 + p, d]
    qt_d = nc.dram_tensor("qt", [128, NPAIR * QBLK], f16, kind="ExternalInput")
    kt_d = nc.dram_tensor("kt", [128, NPAIR * S], f16, kind="ExternalInput")
    vv_d = nc.dram_tensor("vv", [128, B * NKT * D], f16, kind="ExternalInput")
    # out[j][(b%2)*64 + d, q] = out_bqd[2j + b%2, q, d]
    out_d = nc.dram_tensor("out", [NPAIR, 128, QBLK], f32, kind="ExternalOutput")

    with tile.TileContext(nc) as tc, ExitStack() as ctx:
        in_p = ctx.enter_context(tc.tile_pool(name="inp", bufs=1))
        e_p = ctx.enter_context(tc.tile_pool(name="e", bufs=17))
        w_p = ctx.enter_context(tc.tile_pool(name="w", bufs=6))
        t_p = ctx.enter_context(tc.tile_pool(name="tree", bufs=3))
        r_p = ctx.enter_context(tc.tile_pool(name="r", bufs=2))
        st_p = ctx.enter_context(tc.tile_pool(name="stage", bufs=1))
        ps_s = ctx.enter_context(tc.tile_pool(name="ps_s", bufs=2, space="PSUM"))
        ps_o = ctx.enter_context(tc.tile_pool(name="ps_o", bufs=1, space="PSUM"))

        # kt/vv are laid out k-tile-major on the host and DMA'd in per-tile
        # chunks interleaved kt/vv, so tile 0's operands land ~7us in and the
        # loop never waits on later chunks.
        qt = in_p.tile([128, NPAIR * QBLK], f16)
        kt = in_p.tile([128, NKT * NPAIR * KT], f16)
        vv = in_p.tile([128, NKT * B * D], f16)
        CH = NPAIR * KT  # 512 columns per k-tile chunk (for both kt and vv)

        def dma_col(dst, src, c0, c1):
            nc.sync.dma_start(out=dst[:, c0:c1], in_=src[:, c0:c1])

        # Issue order: operands of score pack (t=0, j=0) first, then the
        # rest of tile 0, then per-tile chunks so the loop never waits.
        dma_col(qt, qt_d, 0, QBLK)
        dma_col(kt, kt_d, 0, CH)
        for j in range(1, NPAIR):
            dma_col(qt, qt_d, j * QBLK, (j + 1) * QBLK)
        dma_col(vv, vv_d, 0, CH)
        for t in range(1, NKT):
            dma_col(kt, kt_d, t * CH, (t + 1) * CH)
            dma_col(vv, vv_d, t * CH, (t + 1) * CH)

        # Persistent output accumulators: bank j holds batches 2j (parts
        # 0:64) and 2j+1 (parts 64:128), accumulated over all 32 k-tiles.
        oacc = [
            ps_o.tile([128, QBLK], f32, tag=f"oacc{j}", name=f"oacc{j}")
            for j in range(NPAIR)
        ]

        # AV matmuls pending issue; drained between score packs so PE always
        # services the (ACT-feeding) score matmuls promptly instead of
        # running 16-MM AV bursts that starve ScalarE. Interleaving AV MMs
        # of adjacent k-tiles is safe: psum accumulate-adds commute.
        av_pending = []

        def drain_av(n):
            for _ in range(min(n, len(av_pending))):
                av_pending.pop(0)()

        def emit_scores_exp(t):
            # scores + exp, one 2-bank pack per batch pair
            e_packs = []
            for j in range(NPAIR):
                sc = ps_s.tile([128, 2 * QBLK], f32, tag="sc", name=f"sc{t}_{j}")
                for m in range(2):  # m=0 -> b=2j (rows 0:64), m=1 -> b=2j+1
                    rb = m * 64
                    nc.tensor.matmul(
                        out=sc[:, m * QBLK : (m + 1) * QBLK],
                        lhsT=kt[rb : rb + 64, t * CH + j * KT : t * CH + (j + 1) * KT],
                        rhs=qt[rb : rb + 64, j * QBLK : (j + 1) * QBLK],
                        start=True,
                        stop=True,
                        tile_position=(rb, 0),
                    )
                e = e_p.tile([128, 2 * QBLK], f16, tag="e", name=f"e{t}_{j}")
                # E = exp(scores / sqrt(D)); scores*0.125 in [-6, 6] so no
                # max-subtraction is needed and fp16 range is safe.
                nc.scalar.activation(e[:], sc[:], Exp, scale=0.125)
                e_packs.append(e)
                drain_av(2)
            return e_packs

        def emit_tree(t, e_packs):
            # Z = sum_b E_b, all on DVE. GpSimd is deliberately NOT used:
            # any GpSimd tensor op grabs the shared SBUF port and stretches
            # concurrent DVE tensor_tensor ops ~3x (measured 418ns -> 1370ns).
            # Pack-level adds keep the op count at 5 (2 full-width + 3 half).
            t01 = t_p.tile([128, 2 * QBLK], f16, tag="t01", name=f"t01_{t}")
            nc.vector.tensor_add(t01[:], e_packs[0][:], e_packs[1][:])
            t23 = t_p.tile([128, 2 * QBLK], f16, tag="t23", name=f"t23_{t}")
            nc.vector.tensor_add(t23[:], e_packs[2][:], e_packs[3][:])
            u0 = t_p.tile([128, QBLK], f16, tag="u0", name=f"u0_{t}")
            nc.vector.tensor_add(u0[:], t01[:, :QBLK], t01[:, QBLK:])
            u1 = t_p.tile([128, QBLK], f16, tag="u1", name=f"u1_{t}")
            nc.vector.tensor_add(u1[:], t23[:, :QBLK], t23[:, QBLK:])
            # z tiles of consecutive k-tiles share one [128, 1024] pair tile
            # so ln/exp run once per PAIR at full width (less ACT overhead).
            if t % 2 == 0:
                zp = t_p.tile([128, 2 * QBLK], f16, tag="zp", name=f"zp{t}", bufs=3)
                zpairs[t // 2] = zp
            zp = zpairs[t // 2]
            nc.vector.tensor_add(
                zp[:, (t % 2) * QBLK : (t % 2 + 1) * QBLK], u0[:], u1[:]
            )

        def emit_r_pair(tp):
            # R = 1/Z via exp(-ln(Z)) on ScalarE (shared table set), for a
            # pair of k-tiles (2*tp, 2*tp+1) in one full-width op each.
            zp = zpairs.pop(tp)
            lnz = r_p.tile([128, 2 * QBLK], f32, tag="lnz", name=f"lnz{tp}")
            nc.scalar.activation(lnz[:], zp[:], Ln)
            r16 = r_p.tile([128, 2 * QBLK], f16, tag="r16", name=f"r16_{tp}", bufs=3)
            nc.scalar.activation(r16[:], lnz[:], Exp, scale=-1.0)
            return r16

        def emit_mults(t, e_packs, r16, half):
            # W_b = E_b * R, one fp16 2x-mode op per pack with R broadcast
            # over the pair dimension via a stride-0 access pattern.
            r = r16[:, half * QBLK : (half + 1) * QBLK]
            ws = []
            for j in range(NPAIR):
                w = w_p.tile([128, 2 * QBLK], f16, tag="w", name=f"w{t}_{j}")
                nc.vector.tensor_mul(
                    w[:].rearrange("p (a q) -> p a q", a=2),
                    e_packs[j][:].rearrange("p (a q) -> p a q", a=2),
                    r.rearrange("p (a q) -> p a q", a=1).to_broadcast(
                        (128, 2, QBLK)
                    ),
                )
                ws.append(w)
            return ws

        def emit_av(t, ws):
            # outT_b[d,q] += V_b[t]^T-form matmul, queued for interleaved
            # issue (see drain_av). Reverse order so the first-issued AV's
            # wait (on the last mult's DVE tick) covers the others.
            def mk(b):
                j, m = b // 2, b % 2
                rb = m * 64

                def go():
                    nc.tensor.matmul(
                        out=oacc[j][rb : rb + 64, :],
                        lhsT=vv[:, t * CH + b * D : t * CH + (b + 1) * D],
                        rhs=ws[j][:, m * QBLK : (m + 1) * QBLK],
                        start=(t == 0),
                        stop=(t == NKT - 1),
                        tile_position=(0, rb),
                        skip_group_check=True,
                    )

                return go

            for b in reversed(range(B)):
                av_pending.append(mk(b))

        # Software pipeline with a 2-tile lag between the exp/tree front end
        # and the ln/mult/AV back end: every cross-engine input reaching an
        # engine's strict FIFO was produced >= 2 tiles earlier, so waits are
        # pre-satisfied and each engine streams without head-of-line stalls
        # (a 1-tile lag still stalled ACT ~2us per tile on the add-tree).
        # 2-tile-lag software pipeline: the R/mult/AV back end consumes data
        # produced two k-tiles earlier, so its waits are pre-satisfied when
        # they reach each engine's strict FIFO. The back end runs per PAIR of
        # k-tiles (on odd back-tiles) to use full-width ln/exp ops.
        LAG = 2
        zpairs = {}
        state = {}  # t -> e_packs

        def back_end(tp):
            r16 = emit_r_pair(tp)
            for u, half in ((2 * tp, 0), (2 * tp + 1, 1)):
                ws = emit_mults(u, state.pop(u), r16, half)
                emit_av(u, ws)

        for t in range(NKT + LAG):
            tb = t - LAG
            # Back end runs per pair on odd back-tiles; the final pair is
            # pulled one iteration earlier (its z is ready by then). In the
            # last iterations the back end is emitted BEFORE the scores/exps
            # so its R-ops sit ahead of the final exp block in ScalarE's
            # FIFO and the mults/AVs overlap it instead of trailing it.
            do_back = (
                tb >= 0
                and (tb % 2 == 1 or tb == NKT - 2)
                and (tb | 1) // 2 in zpairs
            )
            if t < NKT:
                e_packs = emit_scores_exp(t)
                emit_tree(t, e_packs)
                state[t] = e_packs
            if do_back:
                back_end((tb | 1) // 2)
        drain_av(len(av_pending))

        # One staging tile + one output DMA (4 separate dma_starts each pay
        # ~2us of setup/completion latency in the kernel tail).
        st = st_p.tile([128, NPAIR * QBLK], f32, tag="st")
        for j in range(NPAIR):
            nc.vector.tensor_copy(
                out=st[:, j * QBLK : (j + 1) * QBLK], in_=oacc[j][:]
            )
        nc.sync.dma_start(
            out=out_d[:].rearrange("j p q -> p j q"),
            in_=st[:].rearrange("p (j q) -> p j q", j=NPAIR),
        )

    return nc


def _get_nc():
    if "nc" not in _cache:
        nc = _build_nc()
        if not nc.is_finalized():
            # Runs Bacc.compile() legalization (wait splitting, reg alloc).
            nc.finalize()
        _cache["nc"] = nc
    return _cache["nc"]


def _host_prep(queries, keys, values):
    """Cast to fp16 and pre-arrange into the SBUF layouts (see _build_nc)."""
    k16 = np.asarray(keys, dtype=np.float16)
    v16 = np.asarray(values, dtype=np.float16)
    q16 = np.asarray(queries, dtype=np.float16)

    # kt[(b%2)*64+d, t*512 + (b//2)*128 + kk] = K[b, t*128+kk, d] (k-tile major)
    kt = np.ascontiguousarray(
        k16.reshape(NPAIR, 2, NKT, KT, D)
        .transpose(1, 4, 2, 0, 3)
        .reshape(128, NKT * NPAIR * KT)
    )
    # vv[p, t*512 + b*64 + d] = V[b, t*128+p, d] (k-tile major)
    vv = np.ascontiguousarray(
        v16.reshape(B, NKT, KT, D).transpose(2, 1, 0, 3).reshape(128, NKT * B * D)
    )

    qts = []
    for c in range(NCORES):
        qc = q16[:, c * QBLK : (c + 1) * QBLK, :]  # [8, 512, 64]
        qt = np.ascontiguousarray(
            qc.transpose(0, 2, 1).reshape(NPAIR, 128, QBLK).transpose(1, 0, 2).reshape(128, NPAIR * QBLK)
        )
        qts.append(qt)
    return qts, kt, vv


def kernel(queries, keys, values):
    global LAST_RESULT
    from concourse.bass_utils import run_bass_kernel_spmd

    queries = np.asarray(queries, dtype=np.float32)
    keys = np.asarray(keys, dtype=np.float32)
    values = np.asarray(values, dtype=np.float32)

    nc = _get_nc()
    qts, kt, vv = _host_prep(queries, keys, values)
    in_maps = [{"qt": qts[c], "kt": kt, "vv": vv} for c in range(NCORES)]

    res = run_bass_kernel_spmd(
        nc,
        in_maps,
        list(range(NCORES)),
        trace=TRACE,
        **TRACE_KWARGS,
    )
    LAST_RESULT = res

    out = np.empty((B, S, D), dtype=np.float32)
    for c in range(NCORES):
        o = res.results[c]["out"]  # [4, 128, 512] = [j, (b%2)*64+d, q]
        out[:, c * QBLK : (c + 1) * QBLK, :] = (
            o.reshape(B, D, QBLK).transpose(0, 2, 1)
        )
    return out
